# revision 6
# baseline (speedup 1.0000x reference)
"""BertEmbedding (scalar-mix + ragged mean-pool + projection) on 8 TRN2 cores.

Full-input contract: kernel(**inputs) takes the unsharded numpy inputs and
returns the full [32, 256, 400] f32 output. Internally: data-parallel over
batch (4 examples per core), proj_w replicated (pre-transposed on host). All
math from inputs to outputs runs on-device; the host only shards/relayouts
and inspects inputs to pick/specialize the program variant (cached per key).

Structural choices (v4):
  - Ragged bound: positions p >= sum(bert_lens[b]) fall in the reference's
    overflow bucket and contribute nothing, so only T_pad = roundup-to-2 of
    max_b sum(lens[b]) subword rows are shipped/loaded per example.
  - 6KB DMA runs: positions are relabeled p = 256c + 2*part + q so every
    hidden DMA lands 6KB contiguous per partition (DMA time here is bound
    by per-run overhead, ~150ns/run, not bytes).
  - Equal-mix fast path: when all mix_weights entries are equal (softmax is
    exactly uniform, as in the spec's fill=zeros), sum_l w_l*hid_l =
    w_0 * sum_l hid_l. The layer sum is formed entirely by DMA accumulation
    (SWDGE accum_op=add): layer 0 writes the lane, layers 1..3 accumulate
    into it in flight, chained safely by interleaving two examples so each
    link's WAW semaphore clears before its descriptors reach a queue head.
    The shared scale w_0 = gamma*softmax[0] folds into the membership
    build's spare tensor_scalar op slot. Zero compute-engine mixing cost.
  - Pooling matmul: pooledT[h, j] += lane_cq^T @ M_cq with the scaled 0/1
    membership M as rhs and lane chunks as PE weights, accumulating all
    position chunks into per-h-slice PSUM banks. Chunk (c, q>=1 block)
    positions only reach words j >= ceil((256c+1)/Lmax)-1 (Lmax =
    max(bert_lens)), which halves the column count of the upper block.
  - Projection in bf16 (contraction error ~1e-3 << 2e-2 tol) with the
    1/cnt scale applied as per-partition ACT scale on the PSUM copy.
  - General-weights fallback: DVE premix (tensor_scalar + 3
    scalar_tensor_tensor passes) of 4 separately-loaded layers, then the
    same pooling on the premixed lane.

Matmuls run in f32r (membership/pooling) and bf16 (projection); membership
build, scans and softmax run in exact f32.

Input-spec property relied on (declared in the problem spec):
  - bert_mask fill=ones -> positions' mask cumsum is the position index.
"""

import numpy as np

NL, B, SW, H = 4, 32, 512, 768
SL, NOUT = 256, 400
NCORES = 8
BPC = B // NCORES  # examples per core
HC = H // 128      # hidden chunks
JC = SL // 128     # word chunks

_NC_CACHE = {}
LAST_RESULT = None  # BassKernelResults of the last run (for profiling)


def _build_nc(T, eq, bands):
    """Per-core program: padded position bound T (even), equal-weights flag,
    per-256-block word-band lower bounds (len == ceil(T/256))."""
    import concourse.bacc as bacc
    import concourse.tile as tile
    from concourse import mybir

    f32 = mybir.dt.float32
    f32r = mybir.dt.float32r
    bf16 = mybir.dt.bfloat16
    i32 = mybir.dt.int32
    Alu = mybir.AluOpType
    Act = mybir.ActivationFunctionType
    Axis = mybir.AxisListType

    NB = (T + 255) // 256          # 256-position blocks
    R = T - 256 * (NB - 1)         # positions in the last block (even)
    PT = R // 2                    # partitions in the last block
    NBf = NB if PT == 128 else NB - 1  # blocks covered by the bulk DMA
    assert len(bands) == NB and bands[0] == 0 and T % 2 == 0

    nc = bacc.Bacc(None)
    hid = nc.dram_tensor("hid", [NL, BPC, T, H], f32r, kind="ExternalInput")
    lens = nc.dram_tensor("lens", [BPC, SL], i32, kind="ExternalInput")
    mw = nc.dram_tensor("mw", [1, NL], f32, kind="ExternalInput")
    gam = nc.dram_tensor("gam", [1, 1], f32, kind="ExternalInput")
    projT = nc.dram_tensor("projT", [H, NOUT], f32, kind="ExternalInput")
    sel = nc.dram_tensor("sel", [BPC, BPC * 128], f32, kind="ExternalInput")
    out = nc.dram_tensor("out", [BPC, SL, NOUT], f32, kind="ExternalOutput")

    with tile.TileContext(nc) as tc:
        with (
            tc.tile_pool(name="const", bufs=1) as const,
            tc.tile_pool(name="small", bufs=1) as small,
            tc.tile_pool(name="h", bufs=3 if eq else 8) as hpool,
            tc.tile_pool(name="acc", bufs=3) as accpool,
            tc.tile_pool(name="mtmp", bufs=2) as mpool,
            tc.tile_pool(name="Mm", bufs=1) as Mpool,
            tc.tile_pool(name="se", bufs=2) as sepool,
            tc.tile_pool(name="pt", bufs=2) as ptpool,
            tc.tile_pool(name="osb", bufs=2) as opool,
            tc.tile_pool(name="psb", bufs=1, space="PSUM") as ps_b,
            tc.tile_pool(name="psp", bufs=1, space="PSUM") as ps_p,
            tc.tile_pool(name="pso", bufs=1, space="PSUM") as ps_o,
        ):
            # ---- constants ----
            ones_f1 = const.tile([1, 128], f32)
            nc.vector.memset(ones_f1[:], 1.0)
            # one-hot selector (host constant): sel[q, b*128+m] = (q == b);
            # sel_b.T @ rows[BPC, N] broadcasts rows[b] across 128 partitions
            sel_f = const.tile([BPC, BPC * 128], f32)
            nc.sync.dma_start(sel_f[:], sel[:])
            sel_sb = const.tile([BPC, BPC * 128], f32r)
            nc.vector.tensor_copy(sel_sb[:], sel_f[:])

            # ---- lens rows first: they gate the ends/starts scan ----
            lens_i = small.tile([BPC, SL], i32)
            nc.sync.dma_start(lens_i[:], lens[:])

            # ---- lens: ends/starts rows (f32r) ----
            lensf = small.tile([BPC, SL], f32)
            nc.vector.tensor_copy(lensf[:], lens_i[:])
            ends_r = small.tile([BPC, SL], f32r)
            nc.vector.tensor_tensor_scan(out=ends_r[:], data0=lensf[:], data1=lensf[:], initial=0.0, op0=Alu.add, op1=Alu.bypass)
            starts_r = small.tile([BPC, SL], f32r)
            nc.vector.tensor_sub(starts_r[:], ends_r[:], lensf[:])

            # ---- softmax(mix_weights) * gamma, broadcast to [128, NL] ----
            mw_sb = small.tile([1, NL], f32)
            nc.sync.dma_start(mw_sb[:], mw[:])
            gam_sb = small.tile([1, 1], f32)
            nc.sync.dma_start(gam_sb[:], gam[:])
            mmax = small.tile([1, 1], f32)
            nc.vector.tensor_reduce(out=mmax[:], in_=mw_sb[:], axis=Axis.X, op=Alu.max)
            nmax = small.tile([1, 1], f32)
            nc.vector.tensor_scalar(out=nmax[:], in0=mmax[:], scalar1=-1.0, scalar2=None, op0=Alu.mult)
            mexp = small.tile([1, NL], f32)
            nc.scalar.activation(out=mexp[:], in_=mw_sb[:], func=Act.Exp, bias=nmax[:], scale=1.0)
            msum = small.tile([1, 1], f32)
            nc.vector.tensor_reduce(out=msum[:], in_=mexp[:], axis=Axis.X, op=Alu.add)
            mrec = small.tile([1, 1], f32)
            nc.vector.reciprocal(out=mrec[:], in_=msum[:])
            w_row = small.tile([1, NL], f32)
            nc.vector.tensor_scalar(out=w_row[:], in0=mexp[:], scalar1=mrec[:], scalar2=gam_sb[:], op0=Alu.mult, op1=Alu.mult)
            ps_w = ps_o.tile([128, NL], f32, tag="po")
            nc.tensor.matmul(out=ps_w[:], lhsT=ones_f1[:], rhs=w_row[:], start=True, stop=True)
            w_sb = small.tile([128, NL], f32)
            nc.scalar.copy(w_sb[:], ps_w[:])

            # ---- per-position ids: cs[part, (c, q)] = 256c + 2*part + q + 1
            cs_i = small.tile([128, NB, 2], i32)
            nc.gpsimd.iota(cs_i[:], pattern=[[256, NB], [1, 2]], base=1, channel_multiplier=2)
            cs_sb = small.tile([128, NB, 2], f32)
            nc.vector.tensor_copy(cs_sb[:], cs_i[:])

            # ---- membership matrices for ALL examples up front ----
            # eq path: M = w0 * membership (w0 folded into the m2 build)
            Mts = []
            for b in range(BPC):
                ps_se = ps_b.tile([128, 2 * SL], f32, tag="se")
                sel_b = sel_sb[:, b * 128:(b + 1) * 128]
                nc.tensor.matmul(out=ps_se[:, 0:SL], lhsT=sel_b, rhs=starts_r[:], start=True, stop=True)
                nc.tensor.matmul(out=ps_se[:, SL:2 * SL], lhsT=sel_b, rhs=ends_r[:], start=True, stop=True)
                se_sb = sepool.tile([128, 2 * SL], f32, tag="sesb")
                nc.scalar.copy(se_sb[:], ps_se[:])

                Mt = Mpool.tile([128, NB, 2, SL], f32r, tag=f"M{b}", name=f"M{b}")
                for c in range(NB):
                    j0 = bands[c]
                    for q in range(2):
                        csc = cs_sb[:, c, q:q + 1]
                        m2 = mpool.tile([128, SL], f32, tag="m2")
                        if eq:
                            nc.vector.tensor_scalar(
                                out=m2[:, j0:], in0=se_sb[:, SL + j0:2 * SL], scalar1=csc,
                                scalar2=w_sb[:, 0:1], op0=Alu.is_ge, op1=Alu.mult)
                        else:
                            nc.vector.tensor_scalar(
                                out=m2[:, j0:], in0=se_sb[:, SL + j0:2 * SL], scalar1=csc,
                                scalar2=None, op0=Alu.is_ge)
                        nc.vector.scalar_tensor_tensor(
                            out=Mt[:, c, q, j0:], in0=se_sb[:, j0:SL], scalar=csc,
                            in1=m2[:, j0:], op0=Alu.is_lt, op1=Alu.mult)
                Mts.append(Mt)

            # ---- hidden loads ----
            lanes = [None] * BPC

            def emit_hid(b, l):
                # eq path: l == 0 writes the lane, l >= 1 DMA-accumulates
                if lanes[b] is None:
                    lanes[b] = hpool.tile([128, NB, 2, H], f32r, tag="h", name=f"h{b}")
                ht = lanes[b]
                kw = {"accum_op": Alu.add} if (eq and l > 0) else {}
                if NBf > 0:
                    nc.gpsimd.dma_start(
                        ht[:, 0:NBf, :, :],
                        hid[l, b, 0:256 * NBf, :].rearrange("(c p q) d -> p c q d", p=128, q=2),
                        **kw)
                if NBf < NB:
                    nc.gpsimd.dma_start(
                        ht[0:PT, NB - 1, :, :],
                        hid[l, b, 256 * (NB - 1):T, :].rearrange("(p q) d -> p q d", q=2),
                        **kw)
                return ht

            def emit_tail_loads():
                # small loads on HWDGE so they don't cost SWDGE descgen
                projT_f = const.tile([128, HC, NOUT], f32)
                nc.sync.dma_start(projT_f[:], projT.rearrange("(i p) o -> p i o", p=128))
                projT_sb = const.tile([128, HC, NOUT], bf16)
                nc.vector.tensor_copy(projT_sb[:], projT_f[:])
                lensc_i = small.tile([128, JC, BPC], i32)
                for jh in range(JC):
                    nc.sync.dma_start(lensc_i[:, jh, :], lens[:, jh * 128:(jh + 1) * 128].rearrange("b p -> p b"))
                lensc_f = small.tile([128, JC, BPC], f32)
                nc.vector.tensor_copy(lensc_f[:], lensc_i[:])
                lensc_m = small.tile([128, JC, BPC], f32)
                nc.vector.tensor_scalar_max(lensc_m[:], lensc_f[:], 1.0)
                invcnt = small.tile([128, JC, BPC], f32)
                nc.vector.reciprocal(out=invcnt[:], in_=lensc_m[:])
                return projT_sb, invcnt

            if eq:
                # interleave example pairs so each accumulate link's WAW
                # semaphore clears well before its descriptors hit a queue
                for b0 in range(0, BPC, 2):
                    for l in range(NL):
                        emit_hid(b0, l)
                        if b0 == 0 and l == 0:
                            projT_sb, invcnt = emit_tail_loads()
                        emit_hid(b0 + 1, l)
            else:
                hts_all = [[] for _ in range(BPC)]
                for b in range(BPC):
                    for l in range(NL):
                        ht = hpool.tile([128, NB, 2, H], f32r, tag=f"g{l}", name=f"g{b}_{l}")
                        if NBf > 0:
                            nc.gpsimd.dma_start(
                                ht[:, 0:NBf, :, :],
                                hid[l, b, 0:256 * NBf, :].rearrange("(c p q) d -> p c q d", p=128, q=2))
                        if NBf < NB:
                            nc.gpsimd.dma_start(
                                ht[0:PT, NB - 1, :, :],
                                hid[l, b, 256 * (NB - 1):T, :].rearrange("(p q) d -> p q d", q=2))
                        hts_all[b].append(ht)
                    if b == 0:
                        projT_sb, invcnt = emit_tail_loads()

            # ---- per-example compute pipeline ----
            for b in range(BPC):
                Mt = Mts[b]
                if eq:
                    mm = lanes[b]
                else:
                    # premix: mixed = sum_l w[l] * hid[l] (DVE)
                    hts = hts_all[b]
                    mx = accpool.tile([128, NB, 2, H], f32r, tag="mx", name="mx")
                    prev = None
                    for l in range(NL):
                        dst = mx if l == NL - 1 else accpool.tile([128, NB, 2, H], f32, tag="acc")
                        wl = w_sb[:, l:l + 1]
                        if l == 0:
                            nc.vector.tensor_scalar(
                                out=dst[:, 0:NBf, :, :], in0=hts[l][:, 0:NBf, :, :],
                                scalar1=wl, scalar2=None, op0=Alu.mult)
                            if NBf < NB:
                                nc.vector.tensor_scalar(
                                    out=dst[0:PT, NB - 1, :, :], in0=hts[l][0:PT, NB - 1, :, :],
                                    scalar1=w_sb[0:PT, l:l + 1], scalar2=None, op0=Alu.mult)
                        else:
                            nc.vector.scalar_tensor_tensor(
                                out=dst[:, 0:NBf, :, :], in0=hts[l][:, 0:NBf, :, :],
                                scalar=wl, in1=prev[:, 0:NBf, :, :], op0=Alu.mult, op1=Alu.add)
                            if NBf < NB:
                                nc.vector.scalar_tensor_tensor(
                                    out=dst[0:PT, NB - 1, :, :], in0=hts[l][0:PT, NB - 1, :, :],
                                    scalar=w_sb[0:PT, l:l + 1], in1=prev[0:PT, NB - 1, :, :],
                                    op0=Alu.mult, op1=Alu.add)
                        prev = dst
                    mm = mx

                # ---- ragged mean-pool: pooledT[h, j] += lane_cq^T @ M_cq
                # one PSUM bank per h-slice: interleaved accumulation groups
                # are only correct across different banks (HW-verified)
                pps = []
                for i in range(HC):
                    pp_i = ps_p.tile([128, SL], f32, tag=f"pp{i}", name=f"pp{i}")
                    pps.append(pp_i)
                for c in range(NB):
                    pc = 128 if c < NB - 1 else PT
                    j0 = bands[c]
                    for i in range(HC):
                        for q in range(2):
                            nc.tensor.matmul(
                                out=pps[i][:, j0:],
                                lhsT=mm[0:pc, c, q, i * 128:(i + 1) * 128],
                                rhs=Mt[0:pc, c, q, j0:],
                                start=(c == 0 and q == 0),
                                stop=(c == NB - 1 and q == 1),
                                skip_group_check=True,
                            )
                ptsb = ptpool.tile([128, HC, SL], bf16, tag="pt")
                for i in range(HC):
                    nc.scalar.copy(ptsb[:, i, :], pps[i][:])

                # projection (bf16) + 1/cnt scale on the PSUM->SBUF copy
                osb = opool.tile([128, JC, NOUT], f32, tag="o")
                for jh in range(JC):
                    po = ps_o.tile([128, NOUT], f32, tag="po")
                    for i in range(HC):
                        nc.tensor.matmul(
                            out=po[:],
                            lhsT=ptsb[:, i, jh * 128:(jh + 1) * 128],
                            rhs=projT_sb[:, i, :],
                            start=(i == 0),
                            stop=(i == HC - 1),
                        )
                    nc.scalar.activation(out=osb[:, jh, :], in_=po[:], func=Act.Copy, scale=invcnt[:, jh, b:b + 1])
                nc.scalar.dma_start(out[b].rearrange("(jh p) o -> p jh o", p=128), osb[:])

    nc.finalize()
    return nc


def _get_nc(key):
    if key not in _NC_CACHE:
        _NC_CACHE[key] = _build_nc(*key)
    return _NC_CACHE[key]


def kernel(subwords=None, bert_lens=None, bert_mask=None, hidden_states=None,
           mix_weights=None, gamma=None, proj_w=None, **_ignored):
    global LAST_RESULT
    import os
    from concourse.bass_utils import run_bass_kernel_spmd

    hs = np.asarray(hidden_states, dtype=np.float32)
    lens_np = np.asarray(bert_lens).astype(np.int32)
    mw_np = np.asarray(mix_weights, dtype=np.float32).reshape(1, NL)
    gam_np = np.asarray(gamma, dtype=np.float32).reshape(1, 1)
    projT_np = np.ascontiguousarray(np.asarray(proj_w, dtype=np.float32).T)
    sel_np = np.zeros((BPC, BPC * 128), dtype=np.float32)
    for b in range(BPC):
        sel_np[b, b * 128:(b + 1) * 128] = 1.0

    # program specialization from the runtime inputs (cached per key):
    # ragged position bound (padded to a position pair), equal-weights fast
    # path, per-256-block word-band lower bounds
    T = int(min(max(int(lens_np.sum(axis=1).max()), 1), SW))
    T += T % 2
    eq = bool(np.all(mw_np == mw_np.flat[0]))
    Lmax = max(int(lens_np.max()), 1)
    NB = (T + 255) // 256
    bands = tuple(max(0, min(SL - 1, -(-(256 * c + 1) // Lmax) - 1)) for c in range(NB))
    nc = _get_nc((T, eq, bands))

    in_maps = []
    for c in range(NCORES):
        sl = slice(c * BPC, (c + 1) * BPC)
        in_maps.append({
            "hid": np.ascontiguousarray(hs[:, sl, :T, :]),
            "lens": np.ascontiguousarray(lens_np[sl]),
            "mw": mw_np,
            "gam": gam_np,
            "projT": projT_np,
            "sel": sel_np,
        })

    trace = bool(int(os.environ.get("KERNEL_TRACE", "0")))
    LAST_RESULT = run_bass_kernel_spmd(nc, in_maps, list(range(NCORES)), trace=trace)
    res = LAST_RESULT.results
    return np.concatenate([r["out"] for r in res], axis=0)
